# revision 1
# baseline (speedup 1.0000x reference)
"""Deformable Conv2d (DCNv2) Trainium2 Bass kernel.

Sharding: 8 cores = (batch b in 0..3) x (image half in 0..1).
Pixel order inside the kernel is x-major: p' = x*64 + yl (yl = local row).
Gather source: per-core vertically-windowed zero-padded image grid
[134 rows x 130 cols], cv row t == global row (r0 + t - 34); col c ==
global col (c - 1).  Doubled along x for pair-gathers:
xd[c, q, 0] = xcv[q-1], xd[c, q, 1] = xcv[q].
"""
import numpy as np
from contextlib import ExitStack

import concourse.bass as bass
import concourse.tile as tile
from concourse import bacc, mybir
from concourse.bass_utils import run_bass_kernel_spmd

B, C, H, W = 4, 128, 128, 128
KK = 9
COUT = 128
N_CORES = 8
HHALF = 64
P = HHALF * W              # 8192
CVX = 130
CVY = 134
NCV = CVY * CVX            # 17420
F32 = mybir.dt.float32
F16 = mybir.dt.float16
I16 = mybir.dt.int16
I32 = mybir.dt.int32

_CACHE = {}


def _build(num_devices=N_CORES):
    AOP = mybir.AluOpType
    nc = bacc.Bacc("TRN2", target_bir_lowering=False, debug=False,
                   enable_asserts=False, num_devices=num_devices)
    xg_ap = nc.dram_tensor("xg", [NCV + 1, 128], F16, kind="ExternalInput").ap()
    xcv_ap = nc.dram_tensor("xcv", [128, NCV], F16, kind="ExternalInput").ap()
    offw_ap = nc.dram_tensor("offw", [128, KK * 27], F16, kind="ExternalInput").ap()
    offb_ap = nc.dram_tensor("offb", [128, 1], F32, kind="ExternalInput").ap()
    wT_ap = nc.dram_tensor("wT", [128, KK * 128], F16, kind="ExternalInput").ap()
    byx_ap = nc.dram_tensor("byx", [128, HHALF * 18], F32, kind="ExternalInput").ap()
    ident_ap = nc.dram_tensor("ident", [128, 128], F32, kind="ExternalInput").ap()
    cst_ap = nc.dram_tensor("cst", [128, 3], F32, kind="ExternalInput").ap()
    out_ap = nc.dram_tensor("out", [128, P], F32, kind="ExternalOutput").ap()
    scr_idx = nc.dram_tensor("scr_idx", [18, P], I16).ap()
    scr_w = nc.dram_tensor("scr_w", [18, 2 * P], F16).ap()

    with tile.TileContext(nc) as tc, ExitStack() as ctx:
        big = ctx.enter_context(tc.tile_pool(name="big", bufs=1))
        pp = ctx.enter_context(tc.tile_pool(name="pp", bufs=1))
        ppost = ctx.enter_context(tc.tile_pool(name="ppost", bufs=1))
        psc = ctx.enter_context(tc.tile_pool(name="psc", bufs=2, space="PSUM"))
        pst = ctx.enter_context(tc.tile_pool(name="pst", bufs=2, space="PSUM"))
        pso = ctx.enter_context(tc.tile_pool(name="pso", bufs=1, space="PSUM"))
        gp = ctx.enter_context(tc.tile_pool(name="gp", bufs=2))
        wbp = ctx.enter_context(tc.tile_pool(name="wbp", bufs=2))
        sgp = ctx.enter_context(tc.tile_pool(name="sgp", bufs=2))
        outp = ctx.enter_context(tc.tile_pool(name="outp", bufs=2))

        xcv = big.tile([128, NCV], F16)
        nc.sync.dma_start(xcv[:], xcv_ap)
        offw = pp.tile([128, KK * 27], F16)
        nc.sync.dma_start(offw[:], offw_ap)
        offb = pp.tile([128, 1], F32)
        nc.sync.dma_start(offb[:], offb_ap)
        wT = pp.tile([128, KK * 128], F16)
        nc.sync.dma_start(wT[:], wT_ap)
        byx = pp.tile([128, HHALF * 18], F32)
        nc.sync.dma_start(byx[:], byx_ap)
        ident = pp.tile([128, 128], F32)
        nc.sync.dma_start(ident[:], ident_ap)
        cst = pp.tile([128, 3], F32)   # [:,0]=qoff, [:,1]=clo, [:,2]=chi
        nc.sync.dma_start(cst[:], cst_ap)


        # ---- Stage 1+2: conv -> transpose -> offT [128(x), 64(yl), 27] ----
        offT = ppost.tile([128, HHALF * 27], F32)
        for t in range(16):
            psc_t = psc.tile([27, 512], F32)
            for k in range(KK):
                ky, kx = k // 3, k % 3
                yl0 = 4 * t
                off_elem = (yl0 + ky + 33) * CVX + kx
                xcva = xcv[:]
                rhs = bass.AP(xcva.tensor, xcva.offset + off_elem,
                              [[xcva.ap[0][0], 128], [CVX, 4], [1, 128]])
                nc.tensor.matmul(psc_t[:], offw[:, k * 27:(k + 1) * 27], rhs,
                                 start=(k == 0), stop=(k == KK - 1))
            ofsb = outp.tile([27, 512], F32, name="ofsb", tag="ofsb")
            nc.vector.tensor_scalar(ofsb[:], psc_t[:],
                                    offb[:27, :], None, op0=AOP.add)
            pst_t = pst.tile([128, 108], F32)
            for j2 in range(4):
                nc.tensor.transpose(pst_t[:, j2 * 27:(j2 + 1) * 27],
                                    ofsb[:27, j2 * 128:(j2 + 1) * 128],
                                    ident[:27, :27])
            nc.vector.tensor_copy(offT[:, t * 108:(t + 1) * 108], pst_t[:])
        offT3 = offT[:].rearrange("p (y c) -> p y c", c=27)

        # ---- Stage 3: offset post-processing ----
        _tn = [0]

        def t1152():
            _tn[0] += 1
            return ppost.tile([128, HHALF * 18], F32, name=f"t1152_{_tn[0]}", tag=f"t1152_{_tn[0]}")

        def t576(dt=F32):
            _tn[0] += 1
            return ppost.tile([128, HHALF * 9], dt, name=f"t576_{_tn[0]}", tag=f"t576_{_tn[0]}")

        pyx = t1152()
        nc.vector.tensor_scalar(pyx[:], offT3[:, :, 0:18], -32.0, 32.0,
                                op0=AOP.max, op1=AOP.min)
        nc.vector.tensor_tensor(pyx[:], pyx[:], byx[:], op=AOP.add)
        msk = t576()
        nc.scalar.activation(msk[:], offT3[:, :, 18:27],
                             mybir.ActivationFunctionType.Sigmoid)
        fi = ppost.tile([128, HHALF * 18], I32)
        nc.vector.tensor_copy(fi[:], pyx[:])
        fl = t1152()
        nc.vector.tensor_copy(fl[:], fi[:])
        cmp = t1152()
        nc.vector.tensor_tensor(cmp[:], fl[:], pyx[:], op=AOP.is_gt)
        nc.vector.tensor_tensor(fl[:], fl[:], cmp[:], op=AOP.subtract)  # floor
        lyx = t1152()
        nc.vector.tensor_tensor(lyx[:], pyx[:], fl[:], op=AOP.subtract)
        omyx = t1152()
        nc.vector.tensor_scalar(omyx[:], lyx[:], -1.0, 1.0,
                                op0=AOP.mult, op1=AOP.add)

        v3 = lambda t: t[:].rearrange("p (y c) -> p y c", c=18)
        y_sl = lambda t: v3(t)[:, :, 0:9]
        x_sl = lambda t: v3(t)[:, :, 9:18]
        fly, flx = y_sl(fl), x_sl(fl)

        # validity masks (y strict; x ring-assisted)
        vtmp = ppost.tile([128, HHALF * 9], F32, name="vtmp", tag="vtmp")

        def vmask(src, lo, hi):
            a = t576()
            nc.vector.tensor_scalar(a[:], src, lo, None, op0=AOP.is_ge)
            nc.vector.tensor_scalar(vtmp[:], src, hi, None, op0=AOP.is_le)
            nc.vector.tensor_tensor(a[:], a[:], vtmp[:], op=AOP.mult)
            return a
        vy0 = vmask(fly, -0.5, 127.5)
        vy1 = vmask(fly, -1.5, 126.5)
        vx0 = vmask(flx, -1.5, 128.5)
        vx1 = vmask(flx, -1.5, 127.5)

        # clamped coords: y0c/y1c in [clo, chi]; x0c in [-1, 128]
        y0c, y1c, x0c = t576(), t576(), t576()
        nc.vector.tensor_scalar(y0c[:], fly, cst[:, 1:2], cst[:, 2:3],
                                op0=AOP.max, op1=AOP.min)
        nc.vector.tensor_scalar(y1c[:], fly, 1.0, None, op0=AOP.add)
        nc.vector.tensor_scalar(y1c[:], y1c[:], cst[:, 1:2], cst[:, 2:3],
                                op0=AOP.max, op1=AOP.min)
        nc.vector.tensor_scalar(x0c[:], flx, -1.0, 128.0, op0=AOP.max, op1=AOP.min)

        # weights
        A0, A1, B0, B1 = t576(), t576(), t576(), t576()
        nc.vector.tensor_tensor(A0[:], y_sl(omyx), vy0[:], op=AOP.mult)
        nc.vector.tensor_tensor(A0[:], A0[:], msk[:], op=AOP.mult)
        nc.vector.tensor_tensor(A1[:], y_sl(lyx), vy1[:], op=AOP.mult)
        nc.vector.tensor_tensor(A1[:], A1[:], msk[:], op=AOP.mult)
        nc.vector.tensor_tensor(B0[:], x_sl(omyx), vx0[:], op=AOP.mult)
        nc.vector.tensor_tensor(B1[:], x_sl(lyx), vx1[:], op=AOP.mult)
        w4 = []
        for Ar in (A0, A1):
            for Bs in (B0, B1):
                nc.vector.tensor_tensor(vtmp[:], Ar[:], Bs[:], op=AOP.mult)
                wh = t576(F16)
                nc.vector.tensor_copy(wh[:], vtmp[:])
                w4.append(wh)          # (r0,s0),(r0,s1),(r1,s0),(r1,s1)
        # gather indices q_r = y_rc*130 + x0c + qoff
        qg = []
        for yc in (y0c, y1c):
            qf = t576()
            nc.vector.scalar_tensor_tensor(qf[:], yc[:], 130.0, x0c[:],
                                           op0=AOP.mult, op1=AOP.add)
            nc.vector.tensor_scalar(qf[:], qf[:], cst[:, 0:1], None, op0=AOP.add)
            qi = t576(I16)
            nc.vector.tensor_copy(qi[:], qf[:])
            qg.append(qi)

        # ---- Stage 4: DRAM roundtrip (x-major order p' = x*64 + yl) ----
        for r in range(2):
            qi3 = qg[r][:].rearrange("p (y c) -> p y c", c=9)
            for k in range(KK):
                dst = bass.AP(scr_idx.tensor, scr_idx.offset + (2 * k + r) * P,
                              [[HHALF, 128], [1, HHALF]])
                nc.sync.dma_start(dst, qi3[:, :, k])
        for r in range(2):
            for s in range(2):
                wh3 = w4[2 * r + s][:].rearrange("p (y c) -> p y c", c=9)
                for k in range(KK):
                    dst = bass.AP(scr_w.tensor,
                                  scr_w.offset + (2 * k + r) * 2 * P + s * P,
                                  [[HHALF, 128], [1, HHALF]])
                    nc.sync.dma_start(dst, wh3[:, :, k])
        idxw = []
        for kr in range(18):
            it = pp.tile([128, P // 16], I16, name=f"idxw{kr}", tag=f"idxw{kr}")
            src = bass.AP(scr_idx.tensor, scr_idx.offset + kr * P,
                          [[1, 16], [16, P // 16]])
            for c8 in range(8):
                nc.sync.dma_start(it[16 * c8:16 * c8 + 16, :], src)
            idxw.append(it)

        # ---- Stage 5: gather + scale + matmul ----
        NCH = 8
        CP = P // NCH              # 1024
        xga = xg_ap
        xrows = bass.AP(xga.tensor, xga.offset, [[128, NCV], [1, 256]])
        for g in range(NCH):
            ps_list = [pso.tile([128, 512], F32, name=f"pso{t2}", tag=f"pso{t2}") for t2 in range(2)]
            for k in range(KK):
                for r in range(2):
                    kr = 2 * k + r
                    gt = gp.tile([128, 2, CP], F16)
                    nc.gpsimd.dma_gather(
                        gt[:], xrows,
                        idxw[kr][:, g * (CP // 16):(g + 1) * (CP // 16)],
                        num_idxs=CP, num_idxs_reg=CP, elem_size=256,
                        elem_step=128, transpose=True, single_packet=False)
                    wb = wbp.tile([128, 2 * CP], F16)
                    src = bass.AP(scr_w.tensor,
                                  scr_w.offset + kr * 2 * P + g * CP,
                                  [[0, 128], [P, 2], [1, CP]])
                    nc.sync.dma_start(wb[:], src)
                    sg = sgp.tile([128, 2 * CP], F16)
                    nc.vector.tensor_tensor(sg[:],
                                            gt[:].rearrange("p a b -> p (a b)"),
                                            wb[:], op=AOP.mult)
                    sga = sg[:]
                    for t2 in range(2):
                        for s in range(2):
                            rhs = bass.AP(sga.tensor,
                                          sga.offset + s * CP + t2 * 512,
                                          [[sga.ap[0][0], 128], [1, 512]])
                            nc.tensor.matmul(
                                ps_list[t2][:], wT[:, k * 128:(k + 1) * 128],
                                rhs, start=(k == 0 and r == 0 and s == 0),
                                stop=(k == KK - 1 and r == 1 and s == 1))
            for t2 in range(2):
                ot = outp.tile([128, 512], F32)
                nc.vector.tensor_copy(ot[:], ps_list[t2][:])
                nc.sync.dma_start(
                    out_ap[:, g * CP + t2 * 512: g * CP + (t2 + 1) * 512],
                    ot[:])
    nc.compile()
    return nc


def _prep_inputs(x, offset_w, offset_b, weight):
    x = np.asarray(x, dtype=np.float32)
    offset_w = np.asarray(offset_w, dtype=np.float32)
    offset_b = np.asarray(offset_b, dtype=np.float32)
    weight = np.asarray(weight, dtype=np.float32)

    remap = np.array([2 * j for j in range(9)] +
                     [2 * j + 1 for j in range(9)] +
                     [18 + j for j in range(9)], dtype=np.int64)
    ow = offset_w[remap]
    ob = offset_b[remap]
    offw = np.zeros((128, KK * 27), dtype=np.float16)
    wT = np.zeros((128, KK * 128), dtype=np.float16)
    for k in range(KK):
        ky, kx = k // 3, k % 3
        offw[:, k * 27:(k + 1) * 27] = ow[:, :, ky, kx].T.astype(np.float16)
        wT[:, k * 128:(k + 1) * 128] = weight[:, :, ky, kx].T.astype(np.float16)
    offb = np.zeros((128, 1), dtype=np.float32)
    offb[:27, 0] = ob
    ident = np.eye(128, dtype=np.float32)

    in_maps = []
    for core in range(N_CORES):
        b, half = core // 2, core % 2
        r0 = half * HHALF
        xcv = np.zeros((128, CVY, CVX), dtype=np.float16)
        for t in range(CVY):
            gr = r0 + t - 34
            if 0 <= gr < H:
                xcv[:, t, 1:129] = x[b, :, gr, :].astype(np.float16)
        xflat = xcv.reshape(128, CVY * CVX)
        xgN = np.zeros((CVY * CVX + 1, 128), dtype=np.float16)
        xgN[:CVY * CVX] = xflat.T
        byx = np.zeros((128, HHALF, 18), dtype=np.float32)
        ylv = np.arange(HHALF)[None, :]
        xv = np.arange(128)[:, None]
        for k in range(KK):
            ky, kx = k // 3, k % 3
            byx[:, :, k] = r0 + ylv + ky - 1       # GLOBAL y base
            byx[:, :, 9 + k] = xv + kx - 1
        cst = np.zeros((128, 3), dtype=np.float32)
        cst[:, 0] = (34 - r0) * CVX + 1            # qoff
        cst[:, 1] = r0 - 34                        # clo
        cst[:, 2] = r0 + 98                        # chi
        in_maps.append({
            "xg": xgN, "xcv": xflat,
            "offw": offw, "offb": offb, "wT": wT,
            "byx": byx.reshape(128, HHALF * 18),
            "ident": ident, "cst": cst,
        })
    return in_maps


def kernel(x, offset_w, offset_b, weight):
    if "nc" not in _CACHE:
        _CACHE["nc"] = _build()
    nc = _CACHE["nc"]
    in_maps = _prep_inputs(x, offset_w, offset_b, weight)
    res = run_bass_kernel_spmd(nc, in_maps, list(range(N_CORES)))
    out = np.zeros((B, COUT, H, W), dtype=np.float32)
    for core in range(N_CORES):
        b, half = core // 2, core % 2
        r0 = half * HHALF
        o = res.results[core]["out"].reshape(COUT, W, HHALF)
        out[b, :, r0:r0 + HHALF, :] = np.transpose(o, (0, 2, 1))
    return out


def _build_null():
    """Same I/O as _build but no compute: for differential timing."""
    nc = bacc.Bacc("TRN2", target_bir_lowering=False, debug=False,
                   enable_asserts=False, num_devices=N_CORES)
    xg_ap = nc.dram_tensor("xg", [NCV + 1, 128], F16, kind="ExternalInput").ap()
    xcv_ap = nc.dram_tensor("xcv", [128, NCV], F16, kind="ExternalInput").ap()
    offw_ap = nc.dram_tensor("offw", [128, KK * 27], F16, kind="ExternalInput").ap()
    offb_ap = nc.dram_tensor("offb", [128, 1], F32, kind="ExternalInput").ap()
    wT_ap = nc.dram_tensor("wT", [128, KK * 128], F16, kind="ExternalInput").ap()
    byx_ap = nc.dram_tensor("byx", [128, HHALF * 18], F32, kind="ExternalInput").ap()
    ident_ap = nc.dram_tensor("ident", [128, 128], F32, kind="ExternalInput").ap()
    cst_ap = nc.dram_tensor("cst", [128, 3], F32, kind="ExternalInput").ap()
    out_ap = nc.dram_tensor("out", [128, P], F32, kind="ExternalOutput").ap()
    with tile.TileContext(nc) as tc, ExitStack() as ctx:
        pool = ctx.enter_context(tc.tile_pool(name="sb", bufs=1))
        t = pool.tile([128, P], F32)
        nc.vector.memset(t[:], 0.0)
        nc.sync.dma_start(out_ap, t[:])
    nc.compile()
    return nc



# revision 13
# speedup vs baseline: 1.7480x; 1.7480x over previous
"""Deformable Conv2d (DCNv2) Trainium2 Bass kernel.

Sharding: 8 cores = (batch b in 0..3) x (image half in 0..1).
Pixel order inside the kernel is x-major: p = x*64 + yl (yl = local row).
Gather source: per-core vertically-windowed zero-padded image grid
[134 rows x 130 cols], xg row t == global row (r0 + t - 34); col c ==
global col (c - 1).  Gather elem = 512B spanning xg rows q,q+1 (the
x-pair of bilinear corners), elem_step=128 (overlapping elements).

v3: bilinear sampling via GPSIMD ap_gather straight out of SBUF
(x kept resident as x-pair-doubled xd [128, NCV, 2]), replacing the
~1us/idx SWDGE dma_gather+DRAM path entirely.  Gather indices are
delivered via PE transposes in SBUF (idx wrap layout built on-chip);
corner weights roundtrip through DRAM once, (i,s)-interleaved, for the
partition-broadcast read.
"""
from concourse import library_config
import numpy as np
from contextlib import ExitStack

import concourse.bass as bass
import concourse.tile as tile
from concourse import bacc, mybir
from concourse.bass_utils import run_bass_kernel_spmd

B, C, H, W = 4, 128, 128, 128
KK = 9
COUT = 128
N_CORES = 8
HHALF = 64
P = HHALF * W              # 8192
CVX = 130
CVY = 134
NCV = CVY * CVX            # 17420
GIDX = 4096                # idxs per ap_gather
NH = P // GIDX
NT = GIDX // 512
F32 = mybir.dt.float32
F16 = mybir.dt.float16
I16 = mybir.dt.int16
I32 = mybir.dt.int32

_CACHE = {}


def _build(num_devices=N_CORES):
    AOP = mybir.AluOpType
    nc = bacc.Bacc("TRN2", target_bir_lowering=False, debug=False,
                   enable_asserts=False, num_devices=num_devices,
                   dynamic_dma_scratch_size=1024)
    xd_ap = nc.dram_tensor("xd", [128, NCV * 2], F16, kind="ExternalInput").ap()
    offw_ap = nc.dram_tensor("offw", [128, KK * 27], F16, kind="ExternalInput").ap()
    offb_ap = nc.dram_tensor("offb", [128, 1], F32, kind="ExternalInput").ap()
    wT_ap = nc.dram_tensor("wT", [128, KK * 128], F16, kind="ExternalInput").ap()
    byx_ap = nc.dram_tensor("byx", [128, HHALF * 18], F32, kind="ExternalInput").ap()
    ident_ap = nc.dram_tensor("ident", [128, 128], F32, kind="ExternalInput").ap()
    cst_ap = nc.dram_tensor("cst", [128, 3], F32, kind="ExternalInput").ap()
    rep16_ap = nc.dram_tensor("rep16", [16, 128], F32, kind="ExternalInput").ap()
    out_ap = nc.dram_tensor("out", [128, P], F32, kind="ExternalOutput").ap()
    scr_w = nc.dram_tensor("scr_w", [KK * 2, 2 * P], F16).ap()

    with tile.TileContext(nc) as tc, ExitStack() as ctx:
        big = ctx.enter_context(tc.tile_pool(name="big", bufs=1))
        pp = ctx.enter_context(tc.tile_pool(name="pp", bufs=1))

        nc.gpsimd.load_library(library_config.ap_gather)
        xd = big.tile([128, NCV, 2], F16)
        nc.sync.dma_start(xd[:].rearrange("p a b -> p (a b)"), xd_ap)
        wT = pp.tile([128, KK * 128], F16)
        nc.sync.dma_start(wT[:], wT_ap)

        # idxw[k*2+r]: wrapped+replicated gather indices, persistent
        idxw = [pp.tile([128, P // 16], I16, name=f"idxw{kr}", tag=f"idxw{kr}")
                for kr in range(18)]

        mid_ctx = ExitStack()
        mid = mid_ctx.enter_context(tc.tile_pool(name="mid", bufs=1))
        offw = mid.tile([128, KK * 27], F16)
        nc.sync.dma_start(offw[:], offw_ap)
        offb = mid.tile([128, 1], F32)
        nc.sync.dma_start(offb[:], offb_ap)
        byx = mid.tile([128, HHALF * 18], F32)
        nc.sync.dma_start(byx[:], byx_ap)
        ident = mid.tile([128, 128], F32)
        nc.sync.dma_start(ident[:], ident_ap)
        cst = mid.tile([128, 3], F32)   # [:,0]=qoff, [:,1]=clo, [:,2]=chi
        nc.sync.dma_start(cst[:], cst_ap)
        rep16 = mid.tile([16, 128], F32)
        nc.sync.dma_start(rep16[:], rep16_ap)

        # ---- Stage 1+2: conv -> transpose -> offT [128(x), 64(yl), 27] ----
        offT = mid.tile([128, HHALF * 27], F32)
        with tc.tile_pool(name="psc", bufs=2, space="PSUM") as psc, \
             tc.tile_pool(name="pst", bufs=2, space="PSUM") as pst, \
             tc.tile_pool(name="cvt", bufs=2) as cvt:
            for t in range(16):
                psc_t = psc.tile([27, 512], F32)
                for k in range(KK):
                    ky, kx = k // 3, k % 3
                    yl0 = 4 * t
                    off_elem = (yl0 + ky + 33) * CVX + kx
                    xcva = xd[:].rearrange("p a b -> p (a b)")
                    rhs = bass.AP(xcva.tensor, xcva.offset + 2 * off_elem,
                                  [[xcva.ap[0][0], 128], [2 * CVX, 4], [2, 128]])
                    nc.tensor.matmul(psc_t[:], offw[:, k * 27:(k + 1) * 27], rhs,
                                     start=(k == 0), stop=(k == KK - 1))
                ofsb = cvt.tile([27, 512], F32, name="ofsb", tag="ofsb")
                nc.vector.tensor_scalar(ofsb[:], psc_t[:],
                                        offb[:27, :], None, op0=AOP.add)
                pst_t = pst.tile([128, 108], F32)
                for j2 in range(4):
                    nc.tensor.transpose(pst_t[:, j2 * 27:(j2 + 1) * 27],
                                        ofsb[:27, j2 * 128:(j2 + 1) * 128],
                                        ident[:27, :27])
                nc.vector.tensor_copy(offT[:, t * 108:(t + 1) * 108], pst_t[:])
        offT3 = offT[:].rearrange("p (y c) -> p y c", c=27)

        # ---- Stage 3: offset post-processing ----
        with tc.tile_pool(name="ppost", bufs=1) as ppost, \
             tc.tile_pool(name="ps4", bufs=2, space="PSUM") as ps4:
            _tn = [0]

            def t1152():
                _tn[0] += 1
                return ppost.tile([128, HHALF * 18], F32,
                                  name=f"t1152_{_tn[0]}", tag=f"t1152_{_tn[0]}")

            def t576(dt=F32):
                _tn[0] += 1
                return ppost.tile([128, HHALF * 9], dt,
                                  name=f"t576_{_tn[0]}", tag=f"t576_{_tn[0]}")

            pyx = t1152()
            nc.vector.tensor_scalar(pyx[:], offT3[:, :, 0:18], -32.0, 32.0,
                                    op0=AOP.max, op1=AOP.min)
            nc.vector.tensor_tensor(pyx[:], pyx[:], byx[:], op=AOP.add)
            msk = t576()
            nc.scalar.activation(msk[:], offT3[:, :, 18:27],
                                 mybir.ActivationFunctionType.Sigmoid)
            fi = ppost.tile([128, HHALF * 18], I32)
            nc.vector.tensor_copy(fi[:], pyx[:])
            fl = t1152()
            nc.vector.tensor_copy(fl[:], fi[:])
            cmp = t1152()
            nc.vector.tensor_tensor(cmp[:], fl[:], pyx[:], op=AOP.is_gt)
            nc.vector.tensor_tensor(fl[:], fl[:], cmp[:], op=AOP.subtract)  # floor
            lyx = t1152()
            nc.vector.tensor_tensor(lyx[:], pyx[:], fl[:], op=AOP.subtract)
            omyx = t1152()
            nc.vector.tensor_scalar(omyx[:], lyx[:], -1.0, 1.0,
                                    op0=AOP.mult, op1=AOP.add)

            v3 = lambda t: t[:].rearrange("p (y c) -> p y c", c=18)
            y_sl = lambda t: v3(t)[:, :, 0:9]
            x_sl = lambda t: v3(t)[:, :, 9:18]
            fly, flx = y_sl(fl), x_sl(fl)

            # validity masks (y strict; x ring-assisted)
            vtmp = ppost.tile([128, HHALF * 9], F32, name="vtmp", tag="vtmp")

            def vmask(src, lo, hi):
                a = t576()
                nc.vector.tensor_scalar(a[:], src, lo, None, op0=AOP.is_ge)
                nc.vector.tensor_scalar(vtmp[:], src, hi, None, op0=AOP.is_le)
                nc.vector.tensor_tensor(a[:], a[:], vtmp[:], op=AOP.mult)
                return a
            vy0 = vmask(fly, -0.5, 127.5)
            vy1 = vmask(fly, -1.5, 126.5)
            vx0 = vmask(flx, -1.5, 128.5)
            vx1 = vmask(flx, -1.5, 127.5)

            # clamped coords: y0c/y1c in [clo, chi]; x0c in [-1, 128]
            y0c, y1c, x0c = t576(), t576(), t576()
            nc.vector.tensor_scalar(y0c[:], fly, cst[:, 1:2], cst[:, 2:3],
                                    op0=AOP.max, op1=AOP.min)
            nc.vector.tensor_scalar(y1c[:], fly, 1.0, None, op0=AOP.add)
            nc.vector.tensor_scalar(y1c[:], y1c[:], cst[:, 1:2], cst[:, 2:3],
                                    op0=AOP.max, op1=AOP.min)
            nc.vector.tensor_scalar(x0c[:], flx, -1.0, 128.0,
                                    op0=AOP.max, op1=AOP.min)

            # weights -> DRAM planes (k, corner c) pixel-major (p = x*64+yl)
            A0, A1, B0, B1 = t576(), t576(), t576(), t576()
            nc.vector.tensor_tensor(A0[:], y_sl(omyx), vy0[:], op=AOP.mult)
            nc.vector.tensor_tensor(A0[:], A0[:], msk[:], op=AOP.mult)
            nc.vector.tensor_tensor(A1[:], y_sl(lyx), vy1[:], op=AOP.mult)
            nc.vector.tensor_tensor(A1[:], A1[:], msk[:], op=AOP.mult)
            nc.vector.tensor_tensor(B0[:], x_sl(omyx), vx0[:], op=AOP.mult)
            nc.vector.tensor_tensor(B1[:], x_sl(lyx), vx1[:], op=AOP.mult)
            for r, Ar in enumerate((A0, A1)):
                wpr = ppost.tile([128, HHALF * 9 * 2], F16,
                                 name=f"wpr{r}", tag=f"wpr{r}")
                wpa = wpr[:]
                for s, Bs in enumerate((B0, B1)):
                    nc.vector.tensor_tensor(vtmp[:], Ar[:], Bs[:], op=AOP.mult)
                    dst = bass.AP(wpa.tensor, wpa.offset + s,
                                  [[wpa.ap[0][0], 128], [2, HHALF * 9]])
                    nc.vector.tensor_copy(dst, vtmp[:])
                # DRAM planes (k,r): addr = x*128 + yl*2 + s
                for k in range(KK):
                    dst = bass.AP(scr_w.tensor,
                                  scr_w.offset + (2 * k + r) * 2 * P,
                                  [[128, 128], [2, HHALF], [1, 2]])
                    srcw = bass.AP(wpa.tensor, wpa.offset + k * 2,
                                   [[wpa.ap[0][0], 128], [18, HHALF], [1, 2]])
                    nc.sync.dma_start(dst, srcw)

            # gather indices q_r = qy2_r*130 + qx2 (qy2 = y_rc+34-r0 <= 162,
            # qx2 = x0c+1 <= 129: exact through PE at any precision),
            # wrapped via PE transposes: idxw[c8*16+j, x*4+u] = q at pixel
            # (x, yl=u*16+j), replicated across the 8 16-partition groups.
            qx2 = t576()
            nc.vector.tensor_scalar(qx2[:], x0c[:], 1.0, None, op0=AOP.add)
            qy2 = [t576(), t576()]
            # cst[:,1] = r0-34, so qy2 = y_rc - cst1
            nc.vector.tensor_scalar(qy2[0][:], y0c[:], cst[:, 1:2], None,
                                    op0=AOP.subtract)
            nc.vector.tensor_scalar(qy2[1][:], y1c[:], cst[:, 1:2], None,
                                    op0=AOP.subtract)

            def trep(srcap, k, u, tag):
                """[128(x), 16(j)] block at (k, u) -> psum [128(j rep), 128(x)]."""
                src = bass.AP(srcap.tensor, srcap.offset + (u * 16) * 9 + k,
                              [[srcap.ap[0][0], 128], [9, 16]])
                pt16 = ps4.tile([16, 128], F32, name=f"pt{tag}", tag=f"pt{tag}")
                nc.tensor.transpose(pt16[:], src, ident[:])
                sb16 = ppost.tile([16, 128], F32, name=f"sb{tag}",
                                  tag=f"sb{tag}")
                nc.vector.tensor_copy(sb16[:], pt16[:])
                pr = ps4.tile([128, 128], F32, name=f"pr{tag}", tag=f"pr{tag}")
                nc.tensor.matmul(pr[:], rep16[:], sb16[:],
                                 start=True, stop=True)
                return pr

            qx2a = qx2[:]
            for k in range(KK):
                for u in range(4):
                    prx = trep(qx2a, k, u, "x")
                    sx = ppost.tile([128, 128], F32, name="sbXr", tag="sbXr")
                    nc.vector.tensor_copy(sx[:], prx[:])
                    for r in range(2):
                        pry = trep(qy2[r][:], k, u, "y")
                        iwa = idxw[2 * k + r][:]
                        dst = bass.AP(iwa.tensor, iwa.offset + u,
                                      [[iwa.ap[0][0], 128], [4, 128]])
                        nc.vector.scalar_tensor_tensor(
                            dst, pry[:], 130.0, sx[:],
                            op0=AOP.mult, op1=AOP.add)

        mid_ctx.close()

        # ---- Stage 4: ap_gather + scale + matmul ----
        with tc.tile_pool(name="gp", bufs=2) as gp, \
             tc.tile_pool(name="wbp", bufs=2) as wbp, \
             tc.tile_pool(name="sgp", bufs=2) as sgp, \
             tc.tile_pool(name="pso", bufs=1, space="PSUM") as pso, \
             tc.tile_pool(name="outp", bufs=2) as outp:
            for h in range(NH):
                ps_list = [pso.tile([128, 512], F32, name=f"pso{t2}",
                                    tag=f"pso{t2}") for t2 in range(NT)]
                for k in range(KK):
                    for r in range(2):
                        kr = 2 * k + r
                        wb = wbp.tile([128, 2 * GIDX], F16)
                        srcw = bass.AP(scr_w.tensor,
                                       scr_w.offset + kr * 2 * P + h * 2 * GIDX,
                                       [[0, 128], [1, 2 * GIDX]])
                        nc.sync.dma_start(wb[:], srcw)
                        gt = gp.tile([128, GIDX, 2], F16)
                        nc.gpsimd.ap_gather(
                            gt[:], xd[:],
                            idxw[kr][:, h * (GIDX // 16):(h + 1) * (GIDX // 16)],
                            channels=128, num_elems=NCV, d=2, num_idxs=GIDX)
                        sg = sgp.tile([128, 2 * GIDX], F16)
                        nc.vector.tensor_tensor(
                            sg[:], gt[:].rearrange("p a b -> p (a b)"),
                            wb[:], op=AOP.mult)
                        sga = sg[:]
                        for s in range(2):
                            for t2 in range(NT):
                                rhs = bass.AP(sga.tensor,
                                              sga.offset + s + 2 * t2 * 512,
                                              [[sga.ap[0][0], 128], [2, 512]])
                                nc.tensor.matmul(
                                    ps_list[t2][:],
                                    wT[:, k * 128:(k + 1) * 128],
                                    rhs, start=(k == 0 and r == 0 and s == 0),
                                    stop=(k == KK - 1 and r == 1 and s == 1))
                for t2 in range(NT):
                    ot = outp.tile([128, 512], F32)
                    nc.vector.tensor_copy(ot[:], ps_list[t2][:])
                    nc.sync.dma_start(
                        out_ap[:, h * GIDX + t2 * 512:h * GIDX + (t2 + 1) * 512],
                        ot[:])
    nc.compile()
    return nc


def _prep_inputs(x, offset_w, offset_b, weight):
    x = np.asarray(x, dtype=np.float32)
    offset_w = np.asarray(offset_w, dtype=np.float32)
    offset_b = np.asarray(offset_b, dtype=np.float32)
    weight = np.asarray(weight, dtype=np.float32)

    remap = np.array([2 * j for j in range(9)] +
                     [2 * j + 1 for j in range(9)] +
                     [18 + j for j in range(9)], dtype=np.int64)
    ow = offset_w[remap]
    ob = offset_b[remap]
    offw = np.zeros((128, KK * 27), dtype=np.float16)
    wT = np.zeros((128, KK * 128), dtype=np.float16)
    for k in range(KK):
        ky, kx = k // 3, k % 3
        offw[:, k * 27:(k + 1) * 27] = ow[:, :, ky, kx].T.astype(np.float16)
        wT[:, k * 128:(k + 1) * 128] = weight[:, :, ky, kx].T.astype(np.float16)
    offb = np.zeros((128, 1), dtype=np.float32)
    offb[:27, 0] = ob
    ident = np.eye(128, dtype=np.float32)

    in_maps = []
    for core in range(N_CORES):
        b, half = core // 2, core % 2
        r0 = half * HHALF
        xcv = np.zeros((128, CVY, CVX), dtype=np.float16)
        for t in range(CVY):
            gr = r0 + t - 34
            if 0 <= gr < H:
                xcv[:, t, 1:129] = x[b, :, gr, :].astype(np.float16)
        xflat = xcv.reshape(128, CVY * CVX)
        xdN = np.zeros((128, NCV, 2), dtype=np.float16)
        xdN[:, :, 0] = xflat
        xdN[:, :-1, 1] = xflat[:, 1:]
        byx = np.zeros((128, HHALF, 18), dtype=np.float32)
        ylv = np.arange(HHALF)[None, :]
        xv = np.arange(128)[:, None]
        for k in range(KK):
            ky, kx = k // 3, k % 3
            byx[:, :, k] = r0 + ylv + ky - 1       # GLOBAL y base
            byx[:, :, 9 + k] = xv + kx - 1
        cst = np.zeros((128, 3), dtype=np.float32)
        cst[:, 0] = (34 - r0) * CVX + 1            # qoff
        cst[:, 1] = r0 - 34                        # clo
        cst[:, 2] = r0 + 98                        # chi
        rep16 = np.zeros((16, 128), dtype=np.float32)
        rep16[np.arange(128) % 16, np.arange(128)] = 1.0
        in_maps.append({
            "xd": xdN.reshape(128, NCV * 2),
            "offw": offw, "offb": offb, "wT": wT,
            "byx": byx.reshape(128, HHALF * 18),
            "ident": ident, "cst": cst, "rep16": rep16,
        })
    return in_maps


def kernel(x, offset_w, offset_b, weight):
    if "nc" not in _CACHE:
        _CACHE["nc"] = _build()
    nc = _CACHE["nc"]
    in_maps = _prep_inputs(x, offset_w, offset_b, weight)
    res = run_bass_kernel_spmd(nc, in_maps, list(range(N_CORES)))
    out = np.zeros((B, COUT, H, W), dtype=np.float32)
    for core in range(N_CORES):
        b, half = core // 2, core % 2
        r0 = half * HHALF
        o = res.results[core]["out"].reshape(COUT, W, HHALF)
        out[b, :, r0:r0 + HHALF, :] = np.transpose(o, (0, 2, 1))
    return out


def _build_null():
    """Same I/O as _build but no compute: for differential timing."""
    nc = bacc.Bacc("TRN2", target_bir_lowering=False, debug=False,
                   enable_asserts=False, num_devices=N_CORES)
    xd_ap = nc.dram_tensor("xd", [128, NCV * 2], F16, kind="ExternalInput").ap()
    offw_ap = nc.dram_tensor("offw", [128, KK * 27], F16, kind="ExternalInput").ap()
    offb_ap = nc.dram_tensor("offb", [128, 1], F32, kind="ExternalInput").ap()
    wT_ap = nc.dram_tensor("wT", [128, KK * 128], F16, kind="ExternalInput").ap()
    byx_ap = nc.dram_tensor("byx", [128, HHALF * 18], F32, kind="ExternalInput").ap()
    ident_ap = nc.dram_tensor("ident", [128, 128], F32, kind="ExternalInput").ap()
    cst_ap = nc.dram_tensor("cst", [128, 3], F32, kind="ExternalInput").ap()
    rep16_ap = nc.dram_tensor("rep16", [16, 128], F32, kind="ExternalInput").ap()
    out_ap = nc.dram_tensor("out", [128, P], F32, kind="ExternalOutput").ap()
    with tile.TileContext(nc) as tc, ExitStack() as ctx:
        pool = ctx.enter_context(tc.tile_pool(name="sb", bufs=1))
        t = pool.tile([128, P], F32)
        nc.vector.memset(t[:], 0.0)
        nc.sync.dma_start(out_ap, t[:])
    nc.compile()
    return nc
